# revision 20
# baseline (speedup 1.0000x reference)
"""Multi-head attention (B=2, S=2048, DM=1024, H=16, DH=64, causal) on 8 TRN2 cores.

Sharding: tensor-parallel over heads. Core c owns heads {2c, 2c+1} = q/k/v dims
[128c, 128c+128). Each core computes its QKV projections, causal attention for
its 2 heads (both batches), and a partial output projection (row-parallel over
Wo). Host unshards by summing the 8 partials and adding bo (the TP all-reduce).

Design (v2 — fully pipelined single pass):
  - xT DMA'd in 8 seq-chunks so QKV compute starts after ~1MB lands.
  - QKV chunks interleaved with attention blocks: block (b,qb) is emitted as
    soon as chunks 0..b*4+qb are in SBUF, so ScalarE exp work starts early.
  - Scores: 2 heads row-packed on the PE (K=64 each, concurrent), f32 PSUM
    (128 keys, 1024 = 2x512 queries).
  - Softmax denominator for h0 FUSED into AV: V2 layout (128 seq, 130) =
    [Vh0 d0-63 | ones | Vh1 d0-63 | pad]; AV h0 lhsT=V2[0:65] -> ctxA[0:65]
    (sum0 at row 64). h1: AV lhsT=V2[65:129] -> ctxB[64:128] plus a 1-col
    ones matmul (lhsT=V2[64:65]) accumulating sum1 into ctxB[0:1] (matmul
    out base partition must be 0/32/64; engines cannot shift partitions).
  - Reciprocal via DVE reciprocal_approx_fast (~5x faster than reciprocal),
    in place at partitions 64 (sum0) / 0 (sum1).
  - rec broadcast across partitions via two K=1 PE matmuls (GpSimd
    partition_broadcast reads the wrong partition on HW; Pool can't see PSUM).
  - Diagonal causal mask via GpSimd affine_select on the exp tile (not DVE).
  - Tail (stash/recip/bcast/mult/outproj) software-pipelined into the NEXT
    attention block's emission so no engine drains.

Causality hardcoded (the reference's attention_mask is always triu causal).
"""

import os
import sys

import numpy as np

try:
    import concourse  # noqa: F401
except ImportError:
    sys.path.insert(0, "/opt/trn_rl_repo")

import ml_dtypes

BF16 = ml_dtypes.bfloat16

B, S, DM = 2, 2048, 1024
H, DH = 16, 64
NCORES = 8
CPC = DM // NCORES  # 128 q/k/v dims per core (2 heads)
BS = B * S  # 4096
Q_W = 512  # query-block width
N_CH = BS // Q_W  # 8 seq chunks
KT_FEAT = DM // 128  # 8 contraction tiles for QKV
NQB = S // Q_W  # 4 query blocks per batch

_CACHE = {}
LAST_EXEC_NS = None
LAST_RESULTS = None


def _build(repeat=1):
    # compat fallbacks for sim/HW divergence bisection (env: BENCH_COMPAT)
    compat = set(os.environ.get("BENCH_COMPAT", "").split(",")) - {""}
    C_MASK = "mask" in compat    # diag mask: DVE cmask mult, not affine_select
    C_EXP = "exp" in compat      # off>0 exp: two activations, not strided AP
    C_SUM = "sum" in compat      # sums_h1: separate PSUM tile (own bank)
    C_RECIP = "recip" in compat  # recip: copy to SBUF + exact reciprocal
    import concourse.mybir as mybir
    from concourse import bacc
    from concourse import tile
    from concourse.masks import make_identity, make_upper_triangular

    f32 = mybir.dt.float32
    f16 = mybir.dt.float16
    bf16 = mybir.dt.bfloat16
    Exp = mybir.ActivationFunctionType.Exp
    Ident = mybir.ActivationFunctionType.Identity
    is_ge = mybir.AluOpType.is_ge

    nc = bacc.Bacc(
        "TRN2",
        target_bir_lowering=False,
        debug=False,
        enable_asserts=False,
        num_devices=NCORES,
    )

    # host supplies xT pre-chunked contiguous: (chunk, 128, feat-tile, 512)
    xT = nc.dram_tensor("xT", (N_CH, 128, KT_FEAT, Q_W), bf16,
                        kind="ExternalInput").ap()
    # weights pre-rearranged host-side to (128, feat-tile, CPC) contiguous
    wq = nc.dram_tensor("wq", (128, KT_FEAT, CPC), bf16,
                        kind="ExternalInput").ap()
    wk = nc.dram_tensor("wk", (128, KT_FEAT, CPC), bf16,
                        kind="ExternalInput").ap()
    wv = nc.dram_tensor("wv", (128, KT_FEAT, CPC), bf16,
                        kind="ExternalInput").ap()
    wo = nc.dram_tensor("wo", (CPC, DM), bf16, kind="ExternalInput").ap()
    bq = nc.dram_tensor("bq", (CPC, 1), f32, kind="ExternalInput").ap()
    bk = nc.dram_tensor("bk", (CPC, 1), f32, kind="ExternalInput").ap()
    bv = nc.dram_tensor("bv", (CPC, 1), f32, kind="ExternalInput").ap()
    out = nc.dram_tensor("out", (BS, DM), f16, kind="ExternalOutput").ap()

    with tile.TileContext(nc) as tc:
      with tc.tile_pool(name="consts", bufs=1) as consts, \
           tc.tile_pool(name="sb", bufs=2) as sb, \
           tc.tile_pool(name="psp", bufs=1, space="PSUM") as psp:

        def body():
            # ---- persistent tiles ------------------------------------------
            wq_sb = consts.tile((128, KT_FEAT, CPC), bf16, name="wq_sb")
            wk_sb = consts.tile((128, KT_FEAT, CPC), bf16, name="wk_sb")
            wv_sb = consts.tile((128, KT_FEAT, CPC), bf16, name="wv_sb")
            wo_sb = consts.tile((CPC, DM), bf16, name="wo_sb")
            bq_sb = consts.tile((CPC, 1), f32, name="bq_sb")
            bk_sb = consts.tile((CPC, 1), f32, name="bk_sb")
            bv_sb = consts.tile((CPC, 1), f32, name="bv_sb")
            # wq first on the same queue as xt0 so Q-proj of chunk 0 (the
            # first PE work) has its deps land earliest
            nc.sync.dma_start(wq_sb[:], wq)
            nc.scalar.dma_start(wk_sb[:], wk)
            nc.scalar.dma_start(wv_sb[:], wv)
            nc.gpsimd.dma_start(wo_sb[:], wo)
            nc.scalar.dma_start(bq_sb[:], bq)
            nc.scalar.dma_start(bk_sb[:], bk)
            nc.scalar.dma_start(bv_sb[:], bv)

            ident_sb = consts.tile((128, 128), bf16, name="ident_sb")
            make_identity(nc, ident_sb[:])
            ones_bf = consts.tile((128, 64), bf16, name="ones_bf")
            nc.vector.memset(ones_bf[:], 1.0)
            if C_MASK:
                cmask = consts.tile((128, 128), bf16, name="cmask")
                make_upper_triangular(nc, cmask[:], val=1.0, diag=True)
            CTX_BUFS = 3 if C_SUM else 2
            O_BUFS = 1 if C_SUM else 2

            QT_sb = consts.tile((128, BS), bf16, name="QT_sb")
            KT_sb = consts.tile((128, BS), bf16, name="KT_sb")
            # V2: (seq 128, 32 k-tiles, 130) = [Vh0 d0-63 | ones | Vh1 d0-63 | pad]
            V2_sb = consts.tile((128, BS // 128, 130), bf16, name="V2_sb")
            nc.vector.memset(V2_sb[:, :, 64:65], 1.0)

            # xT in 8 per-chunk tiles so QKV(ch) only waits its own slab;
            # contiguous host layout + round-robin issue queues so the 8MB
            # input load parallelizes across DMA engines
            dma_engs = [nc.sync, nc.scalar, nc.gpsimd]
            xts = []
            for ch in range(N_CH):
                xt = consts.tile((128, KT_FEAT, Q_W), bf16, name=f"xt{ch}")
                dma_engs[ch % 3].dma_start(xt[:], xT[ch])
                xts.append(xt)

            # ---- QKV chunk -------------------------------------------------
            def emit_qkv(ch):
                c0 = ch * Q_W
                vt = sb.tile((128, Q_W), bf16, name=f"vt{ch}", tag="vt",
                             bufs=2)
                for pname, w_sb, b_sb in (
                    ("q", wq_sb, bq_sb),
                    ("k", wk_sb, bk_sb),
                    ("v", wv_sb, bv_sb),
                ):
                    ps_p = psp.tile((128, Q_W), f32, name=f"ps_{pname}{ch}",
                                    tag="s", bufs=2)
                    for t in range(KT_FEAT):
                        nc.tensor.matmul(
                            ps_p[:],
                            lhsT=w_sb[:, t, :],
                            rhs=xts[ch][:, t, :],
                            start=(t == 0),
                            stop=(t == KT_FEAT - 1),
                        )
                    if pname == "q":
                        nc.scalar.activation(QT_sb[:, c0:c0 + Q_W], ps_p[:],
                                             Ident, bias=b_sb[:])
                    elif pname == "k":
                        nc.scalar.activation(KT_sb[:, c0:c0 + Q_W], ps_p[:],
                                             Ident, bias=b_sb[:])
                    else:
                        nc.scalar.activation(vt[:], ps_p[:], Ident,
                                             bias=b_sb[:])

                # V natural layout via PE transpose; strided copy drops the
                # transposed cols into [0:64] and [65:129] around the ones col
                for sub in range(4):
                    ps_t = psp.tile((128, 128), bf16, name=f"ps_t{ch}_{sub}",
                                    tag="o", bufs=O_BUFS)
                    nc.tensor.transpose(
                        ps_t[:],
                        vt[:, sub * 128:(sub + 1) * 128],
                        ident_sb[:],
                    )
                    ti = ch * 4 + sub
                    dst = V2_sb[:, ti, :].rearrange(
                        "p (a b) -> p a b", b=65)[:, :, 0:64]
                    src = ps_t[:].rearrange("p (a b) -> p a b", b=64)
                    nc.vector.tensor_copy(dst, src)

            # ---- attention block (b, qb) with pipelined tail ---------------
            def make_tail(b, qb, ctxA, ctxB, ctxS):
                g0 = b * S + qb * Q_W
                hold = {}

                def stash():
                    cu = sb.tile((128, Q_W), bf16, name=f"cu{b}_{qb}",
                                 tag="cu", bufs=2)
                    nc.vector.tensor_copy(cu[0:64, :], ctxA[0:64, :])
                    nc.vector.tensor_copy(cu[64:128, :], ctxB[64:128, :])
                    hold["cu"] = cu

                def recips():
                    st = sb.tile((128, Q_W), f32, name=f"st{b}_{qb}",
                                 tag="st", bufs=2)
                    # custom-DVE ops (reciprocal_approx_*) silently misread
                    # on HW when the AP base partition != 0, and can't read
                    # PSUM: bounce sums rows into a memset SBUF tile and do
                    # ONE full-tile recip at base 0 (cost is free-size-driven)
                    sti = sb.tile((128, Q_W), f32, name=f"sti{b}_{qb}",
                                  tag="sti", bufs=2)
                    nc.vector.memset(sti[:], 1.0)
                    nc.vector.tensor_copy(sti[64:65, :], ctxA[64:65, :])
                    nc.vector.tensor_copy(sti[0:1, :], ctxS[0:1, :])
                    if C_RECIP:
                        nc.vector.reciprocal(st[64:65, :], sti[64:65, :])
                        nc.vector.reciprocal(st[0:1, :], sti[0:1, :])
                    else:
                        nc.vector.reciprocal_approx_fast(st[:], sti[:])
                    stb = sb.tile((128, Q_W), bf16, name=f"stb{b}_{qb}",
                                  tag="stb", bufs=2)
                    if C_RECIP:
                        nc.vector.tensor_copy(stb[64:65, :], st[64:65, :])
                        nc.vector.tensor_copy(stb[0:1, :], st[0:1, :])
                    else:
                        nc.vector.tensor_copy(stb[:], st[:])
                    hold["stb"] = stb

                def bcast_mult():
                    stb = hold["stb"]
                    ps_bc = psp.tile((128, Q_W), f32, name=f"ps_bc{b}_{qb}",
                                     tag="o", bufs=O_BUFS)
                    nc.tensor.matmul(ps_bc[0:64, :],
                                     lhsT=ones_bf[64:65, 0:64],
                                     rhs=stb[64:65, :])
                    nc.tensor.matmul(ps_bc[64:128, :],
                                     lhsT=ones_bf[0:1, 0:64],
                                     rhs=stb[0:1, :])
                    cx = sb.tile((128, Q_W), bf16, name=f"cx{b}_{qb}",
                                 tag="cx", bufs=2)
                    nc.vector.tensor_mul(cx[:], hold["cu"][:], ps_bc[:])
                    hold["cx"] = cx

                def outproj(k):
                    cx = hold["cx"]
                    for sub in (2 * k, 2 * k + 1):
                        o_sb = sb.tile((128, DM), f16,
                                       name=f"o{b}_{qb}_{sub}",
                                       tag="ob", bufs=3)
                        for nn in range(2):
                            ps_o = psp.tile((128, 512), f32,
                                            name=f"ps_o{b}_{qb}_{sub}_{nn}",
                                            tag="o", bufs=O_BUFS)
                            nc.tensor.matmul(
                                ps_o[:],
                                lhsT=cx[:, sub * 128:(sub + 1) * 128],
                                rhs=wo_sb[:, nn * 512:(nn + 1) * 512],
                            )
                            nc.vector.tensor_copy(
                                o_sb[:, nn * 512:(nn + 1) * 512], ps_o[:])
                        r0 = g0 + sub * 128
                        nc.sync.dma_start(out[r0:r0 + 128, :], o_sb[:])

                return [stash, recips, bcast_mult,
                        lambda: outproj(0), lambda: outproj(1)]

            def emit_attn(b, qb, pend):
                qb0 = qb * Q_W
                g0 = b * S + qb0
                n_t = (qb0 + Q_W) // 128  # causal: k-tiles needed
                ctxA = psp.tile((128, Q_W), f32, name=f"ctxA{b}_{qb}",
                                tag="ctx", bufs=CTX_BUFS)
                ctxB = psp.tile((128, Q_W), f32, name=f"ctxB{b}_{qb}",
                                tag="ctx", bufs=CTX_BUFS)
                ctxS = (psp.tile((128, Q_W), f32, name=f"ctxS{b}_{qb}",
                                 tag="ctx", bufs=CTX_BUFS) if C_SUM else ctxB)

                avq = []  # delayed AV args: (exp_sb, t, off, w)

                def emit_av(exp_sb, t, off, w):
                    first = t == 0
                    last = t == n_t - 1
                    kti = (b * S + 128 * t) // 128
                    nc.tensor.matmul(
                        ctxA[0:65, off:Q_W],
                        lhsT=V2_sb[:, kti, 0:65],
                        rhs=exp_sb[:, 0:w],
                        start=first,
                        stop=last,
                    )
                    nc.tensor.matmul(
                        ctxB[64:128, off:Q_W],
                        lhsT=V2_sb[:, kti, 65:129],
                        rhs=exp_sb[:, 512:512 + w],
                        start=first,
                        stop=last,
                        skip_group_check=True,
                    )
                    nc.tensor.matmul(
                        ctxS[0:1, off:Q_W],
                        lhsT=V2_sb[:, kti, 64:65],
                        rhs=exp_sb[:, 512:512 + w],
                        start=first,
                        stop=last,
                        skip_group_check=True,
                    )

                for t in range(n_t):
                    k0 = 128 * t
                    off = max(0, k0 - qb0)
                    w = Q_W - off
                    diag = k0 >= qb0
                    ps_s = psp.tile((128, 1024), f32,
                                    name=f"ps_s{b}_{qb}_{t}",
                                    tag="s", bufs=2)
                    exp_sb = sb.tile((128, 1024), bf16,
                                     name=f"exp{b}_{qb}_{t}",
                                     tag="exp", bufs=4)
                    for h in range(2):
                        nc.tensor.matmul(
                            ps_s[:, h * 512:h * 512 + w],
                            lhsT=KT_sb[h * 64:(h + 1) * 64,
                                       b * S + k0:b * S + k0 + 128],
                            rhs=QT_sb[h * 64:(h + 1) * 64,
                                      g0 + off:g0 + Q_W],
                            start=True,
                            stop=True,
                            tile_position=(h * 64, 0),
                            skip_group_check=True,
                        )
                    if t == 0 and pend:
                        pend[0]()  # stash(prev): frees ctx ring slots
                    if off == 0:
                        nc.scalar.activation(exp_sb[:, :1024], ps_s[:, :1024],
                                             Exp, scale=0.125)
                    elif C_EXP:
                        nc.scalar.activation(exp_sb[:, 0:w], ps_s[:, 0:w],
                                             Exp, scale=0.125)
                        nc.scalar.activation(exp_sb[:, 512:512 + w],
                                             ps_s[:, 512:512 + w],
                                             Exp, scale=0.125)
                    else:
                        s3 = ps_s[:].rearrange("p (h q) -> p h q",
                                               h=2)[:, :, 0:w]
                        e3 = exp_sb[:].rearrange("p (h q) -> p h q",
                                                 h=2)[:, :, 0:w]
                        nc.scalar.activation(e3, s3, Exp, scale=0.125)
                    if t == 0 and pend:
                        pend[1]()  # recips(prev): last ctx(prev) readers
                    if diag and C_MASK:
                        for h in range(2):
                            sb0 = h * 512
                            nc.vector.tensor_mul(
                                exp_sb[:, sb0:sb0 + 128],
                                exp_sb[:, sb0:sb0 + 128],
                                cmask[:],
                            )
                    elif diag:
                        em = exp_sb[:].rearrange("p (h q) -> p h q",
                                                 h=2)[:, :, 0:128]
                        nc.gpsimd.affine_select(
                            em, em,
                            pattern=[[0, 2], [1, 128]],
                            compare_op=is_ge,
                            fill=0.0,
                            base=0,
                            channel_multiplier=-1,
                        )
                    if t >= 1:
                        emit_av(*avq[t - 1])
                    avq.append((exp_sb, t, off, w))
                    if t == 1 and pend:
                        pend[2]()  # bcast+mult(prev)
                    if t == 2 and pend:
                        pend[3]()  # outproj(prev) subs 0-1
                    if t == 3 and pend:
                        pend[4]()  # outproj(prev) subs 2-3
                emit_av(*avq[n_t - 1])
                return make_tail(b, qb, ctxA, ctxB, ctxS)

            # ---- interleaved emission --------------------------------------
            done_ch = 0
            pend = []
            for b in range(B):
                for qb in range(NQB):
                    need = b * NQB + qb + 1
                    while done_ch < need:
                        emit_qkv(done_ch)
                        done_ch += 1
                    pend = emit_attn(b, qb, pend)
            while done_ch < N_CH:
                emit_qkv(done_ch)
                done_ch += 1
            for step in pend:
                step()

        if repeat == 1:
            body()
        else:
            with tc.For_i(0, repeat, 1):
                body()

    nc.compile()
    return nc


def _prep_inputs(x, Wq, bq, Wk, bk, Wv, bv, Wo):
    """Build the 8 per-core input maps (host-side sharding)."""
    x = np.asarray(x, dtype=np.float32)
    xT = x.reshape(BS, DM).T.astype(BF16)  # (DM, BS)
    # (chunk, 128, feat-tile, 512): xTc[ch, p, t, q] = xT[t*128+p, ch*512+q]
    xTc = np.ascontiguousarray(
        xT.reshape(KT_FEAT, 128, N_CH, Q_W).transpose(2, 1, 0, 3))

    def _w(W, sl):  # (128, feat-tile, CPC): w[p, t, m] = W[sl][m, t*128+p]
        wT = np.asarray(W, np.float32)[sl, :].T.astype(BF16)  # (DM, CPC)
        return np.ascontiguousarray(
            wT.reshape(KT_FEAT, 128, CPC).transpose(1, 0, 2))

    in_maps = []
    for c in range(NCORES):
        sl = slice(c * CPC, (c + 1) * CPC)
        in_maps.append({
            "xT": xTc,
            "wq": _w(Wq, sl),
            "wk": _w(Wk, sl),
            "wv": _w(Wv, sl),
            "wo": np.ascontiguousarray(np.asarray(Wo, np.float32)[:, sl].T).astype(BF16),
            "bq": np.asarray(bq, np.float32)[sl].reshape(CPC, 1).copy(),
            "bk": np.asarray(bk, np.float32)[sl].reshape(CPC, 1).copy(),
            "bv": np.asarray(bv, np.float32)[sl].reshape(CPC, 1).copy(),
        })
    return in_maps


def _run(in_maps, trace=False):
    global LAST_EXEC_NS, LAST_RESULTS
    from concourse import bass_utils

    if "nc" not in _CACHE:
        _CACHE["nc"] = _build()
    nc = _CACHE["nc"]
    res = bass_utils.run_bass_kernel_spmd(
        nc, in_maps, core_ids=list(range(NCORES)), trace=trace,
    )
    LAST_EXEC_NS = getattr(res, "exec_time_ns", None)
    LAST_RESULTS = res
    return res.results


def kernel(x, Wq, bq, Wk, bk, Wv, bv, Wo, bo, attention_mask=None, _trace=False):
    """Full inputs in, full output out. attention_mask is the reference's
    causal mask; causality is hardcoded in the kernel."""
    in_maps = _prep_inputs(x, Wq, bq, Wk, bk, Wv, bv, Wo)
    results = _run(in_maps, trace=_trace)
    acc = np.zeros((BS, DM), dtype=np.float32)
    for c in range(NCORES):
        acc += results[c]["out"].astype(np.float32)
    acc += np.asarray(bo, np.float32)[None, :]
    return acc.reshape(B, S, DM)


# revision 22
# speedup vs baseline: 1.0164x; 1.0164x over previous
"""Multi-head attention (B=2, S=2048, DM=1024, H=16, DH=64, causal) on 8 TRN2 cores.

Sharding: tensor-parallel over heads. Core c owns heads {2c, 2c+1} = q/k/v dims
[128c, 128c+128). Each core computes its QKV projections, causal attention for
its 2 heads (both batches), and a partial output projection (row-parallel over
Wo). Host unshards by summing the 8 partials and adding bo (the TP all-reduce).

Design (v2 — fully pipelined single pass):
  - xT DMA'd in 8 seq-chunks so QKV compute starts after ~1MB lands.
  - QKV chunks interleaved with attention blocks: block (b,qb) is emitted as
    soon as chunks 0..b*4+qb are in SBUF, so ScalarE exp work starts early.
  - Scores: 2 heads row-packed on the PE (K=64 each, concurrent), f32 PSUM
    (128 keys, 1024 = 2x512 queries).
  - Softmax denominator for h0 FUSED into AV: V2 layout (128 seq, 130) =
    [Vh0 d0-63 | ones | Vh1 d0-63 | pad]; AV h0 lhsT=V2[0:65] -> ctxA[0:65]
    (sum0 at row 64). h1: AV lhsT=V2[65:129] -> ctxB[64:128] plus a 1-col
    ones matmul (lhsT=V2[64:65]) accumulating sum1 into ctxB[0:1] (matmul
    out base partition must be 0/32/64; engines cannot shift partitions).
  - Reciprocal via DVE reciprocal_approx_fast (~5x faster than reciprocal),
    in place at partitions 64 (sum0) / 0 (sum1).
  - rec broadcast across partitions via two K=1 PE matmuls (GpSimd
    partition_broadcast reads the wrong partition on HW; Pool can't see PSUM).
  - Diagonal causal mask via GpSimd affine_select on the exp tile (not DVE).
  - Tail (stash/recip/bcast/mult/outproj) software-pipelined into the NEXT
    attention block's emission so no engine drains.

Causality hardcoded (the reference's attention_mask is always triu causal).
"""

import os
import sys

import numpy as np

try:
    import concourse  # noqa: F401
except ImportError:
    sys.path.insert(0, "/opt/trn_rl_repo")

import ml_dtypes

BF16 = ml_dtypes.bfloat16

B, S, DM = 2, 2048, 1024
H, DH = 16, 64
NCORES = 8
CPC = DM // NCORES  # 128 q/k/v dims per core (2 heads)
BS = B * S  # 4096
Q_W = 512  # query-block width
N_CH = BS // Q_W  # 8 seq chunks
KT_FEAT = DM // 128  # 8 contraction tiles for QKV
NQB = S // Q_W  # 4 query blocks per batch

_CACHE = {}
LAST_EXEC_NS = None
LAST_RESULTS = None


def _build(repeat=1):
    # compat fallbacks for sim/HW divergence bisection (env: BENCH_COMPAT)
    compat = set(os.environ.get("BENCH_COMPAT", "").split(",")) - {""}
    C_MASK = "mask" in compat    # diag mask: DVE cmask mult, not affine_select
    C_EXP = "exp" in compat      # off>0 exp: two activations, not strided AP
    C_SUM = "sum" in compat      # sums_h1: separate PSUM tile (own bank)
    C_RECIP = "recip" in compat  # recip: copy to SBUF + exact reciprocal
    import concourse.mybir as mybir
    from concourse import bacc
    from concourse import tile
    from concourse.masks import make_identity, make_upper_triangular

    f32 = mybir.dt.float32
    f16 = mybir.dt.float16
    bf16 = mybir.dt.bfloat16
    Exp = mybir.ActivationFunctionType.Exp
    Ident = mybir.ActivationFunctionType.Identity
    is_ge = mybir.AluOpType.is_ge

    nc = bacc.Bacc(
        "TRN2",
        target_bir_lowering=False,
        debug=False,
        enable_asserts=False,
        num_devices=NCORES,
    )

    # host supplies xT pre-chunked contiguous: (chunk, 128, feat-tile, 512)
    xT = nc.dram_tensor("xT", (N_CH, 128, KT_FEAT, Q_W), bf16,
                        kind="ExternalInput").ap()
    # weights pre-rearranged host-side to (128, feat-tile, CPC) contiguous
    wq = nc.dram_tensor("wq", (128, KT_FEAT, CPC), bf16,
                        kind="ExternalInput").ap()
    wk = nc.dram_tensor("wk", (128, KT_FEAT, CPC), bf16,
                        kind="ExternalInput").ap()
    wv = nc.dram_tensor("wv", (128, KT_FEAT, CPC), bf16,
                        kind="ExternalInput").ap()
    wo = nc.dram_tensor("wo", (CPC, DM), bf16, kind="ExternalInput").ap()
    bq = nc.dram_tensor("bq", (CPC, 1), f32, kind="ExternalInput").ap()
    bk = nc.dram_tensor("bk", (CPC, 1), f32, kind="ExternalInput").ap()
    bv = nc.dram_tensor("bv", (CPC, 1), f32, kind="ExternalInput").ap()
    out = nc.dram_tensor("out", (BS, DM), f16, kind="ExternalOutput").ap()

    with tile.TileContext(nc) as tc:
      with tc.tile_pool(name="consts", bufs=1) as consts, \
           tc.tile_pool(name="sb", bufs=2) as sb, \
           tc.tile_pool(name="psp", bufs=1, space="PSUM") as psp:

        def body():
            # ---- persistent tiles ------------------------------------------
            wq_sb = consts.tile((128, KT_FEAT, CPC), bf16, name="wq_sb")
            wk_sb = consts.tile((128, KT_FEAT, CPC), bf16, name="wk_sb")
            wv_sb = consts.tile((128, KT_FEAT, CPC), bf16, name="wv_sb")
            wo_sb = consts.tile((CPC, DM), bf16, name="wo_sb")
            bq_sb = consts.tile((CPC, 1), f32, name="bq_sb")
            bk_sb = consts.tile((CPC, 1), f32, name="bk_sb")
            bv_sb = consts.tile((CPC, 1), f32, name="bv_sb")
            # wq first on the same queue as xt0 so Q-proj of chunk 0 (the
            # first PE work) has its deps land earliest
            nc.sync.dma_start(wq_sb[:], wq)
            nc.scalar.dma_start(wk_sb[:], wk)
            nc.scalar.dma_start(wv_sb[:], wv)
            nc.gpsimd.dma_start(wo_sb[:], wo)
            nc.scalar.dma_start(bq_sb[:], bq)
            nc.scalar.dma_start(bk_sb[:], bk)
            nc.scalar.dma_start(bv_sb[:], bv)

            ident_sb = consts.tile((128, 128), bf16, name="ident_sb")
            make_identity(nc, ident_sb[:])
            ones_bf = consts.tile((128, 64), bf16, name="ones_bf")
            nc.vector.memset(ones_bf[:], 1.0)
            if C_MASK:
                cmask = consts.tile((128, 128), bf16, name="cmask")
                make_upper_triangular(nc, cmask[:], val=1.0, diag=True)
            CTX_BUFS = 3 if C_SUM else 2
            O_BUFS = 1 if C_SUM else 2

            QT_sb = consts.tile((128, BS), bf16, name="QT_sb")
            KT_sb = consts.tile((128, BS), bf16, name="KT_sb")
            # V2: (seq 128, 32 k-tiles, 130) = [Vh0 d0-63 | ones | Vh1 d0-63 | pad]
            V2_sb = consts.tile((128, BS // 128, 130), bf16, name="V2_sb")
            nc.vector.memset(V2_sb[:, :, 64:65], 1.0)

            # xT in 8 per-chunk tiles so QKV(ch) only waits its own slab;
            # contiguous host layout + round-robin issue queues so the 8MB
            # input load parallelizes across DMA engines
            dma_engs = [nc.sync, nc.scalar, nc.gpsimd]
            xts = []
            for ch in range(N_CH):
                xt = consts.tile((128, KT_FEAT, Q_W), bf16, name=f"xt{ch}")
                if ch == 0:
                    # per-feature-tile DMAs: the first Q-proj matmul only
                    # waits 128KB instead of the whole 1MB chunk
                    for t in range(KT_FEAT):
                        nc.sync.dma_start(xt[:, t:t + 1, :], xT[0, :, t:t + 1])
                else:
                    dma_engs[ch % 3].dma_start(xt[:], xT[ch])
                xts.append(xt)

            # persistent pre-memset recip staging (ping-pong): full-tile
            # reciprocal_approx_fast needs every partition initialized, and
            # memset in the per-block chain would delay PSUM release
            stis = []
            for i in range(2):
                sti = consts.tile((128, Q_W), f32, name=f"sti{i}")
                nc.vector.memset(sti[:], 1.0)
                stis.append(sti)

            # ---- QKV chunk -------------------------------------------------
            def emit_qkv(ch):
                c0 = ch * Q_W
                vt = sb.tile((128, Q_W), bf16, name=f"vt{ch}", tag="vt",
                             bufs=2)
                for pname, w_sb, b_sb in (
                    ("q", wq_sb, bq_sb),
                    ("k", wk_sb, bk_sb),
                    ("v", wv_sb, bv_sb),
                ):
                    ps_p = psp.tile((128, Q_W), f32, name=f"ps_{pname}{ch}",
                                    tag="s", bufs=2)
                    for t in range(KT_FEAT):
                        nc.tensor.matmul(
                            ps_p[:],
                            lhsT=w_sb[:, t, :],
                            rhs=xts[ch][:, t, :],
                            start=(t == 0),
                            stop=(t == KT_FEAT - 1),
                        )
                    if pname == "q":
                        nc.scalar.activation(QT_sb[:, c0:c0 + Q_W], ps_p[:],
                                             Ident, bias=b_sb[:])
                    elif pname == "k":
                        nc.scalar.activation(KT_sb[:, c0:c0 + Q_W], ps_p[:],
                                             Ident, bias=b_sb[:])
                    else:
                        nc.scalar.activation(vt[:], ps_p[:], Ident,
                                             bias=b_sb[:])

                # V natural layout via PE transpose; strided copy drops the
                # transposed cols into [0:64] and [65:129] around the ones col
                for sub in range(4):
                    ps_t = psp.tile((128, 128), bf16, name=f"ps_t{ch}_{sub}",
                                    tag="o", bufs=O_BUFS)
                    nc.tensor.transpose(
                        ps_t[:],
                        vt[:, sub * 128:(sub + 1) * 128],
                        ident_sb[:],
                    )
                    ti = ch * 4 + sub
                    dst = V2_sb[:, ti, :].rearrange(
                        "p (a b) -> p a b", b=65)[:, :, 0:64]
                    src = ps_t[:].rearrange("p (a b) -> p a b", b=64)
                    nc.vector.tensor_copy(dst, src)

            # ---- attention block (b, qb) with pipelined tail ---------------
            def make_tail(b, qb, ctxA, ctxB, ctxS, blk):
                g0 = b * S + qb * Q_W
                hold = {}

                def stash():
                    # on ScalarE: keeps the DVE queue free so recips() can
                    # release the ctx PSUM ring with minimum latency
                    cu = sb.tile((128, Q_W), bf16, name=f"cu{b}_{qb}",
                                 tag="cu", bufs=2)
                    nc.scalar.copy(cu[0:64, :], ctxA[0:64, :])
                    nc.scalar.copy(cu[64:128, :], ctxB[64:128, :])
                    hold["cu"] = cu

                def recips():
                    st = sb.tile((128, Q_W), f32, name=f"st{b}_{qb}",
                                 tag="st", bufs=2)
                    # custom-DVE ops (reciprocal_approx_*) silently misread
                    # on HW when the AP base partition != 0, and can't read
                    # PSUM: bounce sums rows into a pre-memset SBUF tile and
                    # do ONE full-tile recip at base 0 (free-size-driven cost)
                    sti = stis[blk % 2]
                    nc.vector.tensor_copy(sti[64:65, :], ctxA[64:65, :])
                    nc.vector.tensor_copy(sti[0:1, :], ctxS[0:1, :])
                    if C_RECIP:
                        nc.vector.reciprocal(st[64:65, :], sti[64:65, :])
                        nc.vector.reciprocal(st[0:1, :], sti[0:1, :])
                    else:
                        nc.vector.reciprocal_approx_fast(st[:], sti[:])
                    stb = sb.tile((128, Q_W), bf16, name=f"stb{b}_{qb}",
                                  tag="stb", bufs=2)
                    if C_RECIP:
                        nc.vector.tensor_copy(stb[64:65, :], st[64:65, :])
                        nc.vector.tensor_copy(stb[0:1, :], st[0:1, :])
                    else:
                        nc.vector.tensor_copy(stb[:], st[:])
                    hold["stb"] = stb

                def bcast_mult():
                    stb = hold["stb"]
                    ps_bc = psp.tile((128, Q_W), f32, name=f"ps_bc{b}_{qb}",
                                     tag="o", bufs=O_BUFS)
                    nc.tensor.matmul(ps_bc[0:64, :],
                                     lhsT=ones_bf[64:65, 0:64],
                                     rhs=stb[64:65, :])
                    nc.tensor.matmul(ps_bc[64:128, :],
                                     lhsT=ones_bf[0:1, 0:64],
                                     rhs=stb[0:1, :])
                    cx = sb.tile((128, Q_W), bf16, name=f"cx{b}_{qb}",
                                 tag="cx", bufs=2)
                    nc.vector.tensor_mul(cx[:], hold["cu"][:], ps_bc[:])
                    hold["cx"] = cx

                def outproj(k):
                    cx = hold["cx"]
                    for sub in (2 * k, 2 * k + 1):
                        o_sb = sb.tile((128, DM), f16,
                                       name=f"o{b}_{qb}_{sub}",
                                       tag="ob", bufs=3)
                        for nn in range(2):
                            ps_o = psp.tile((128, 512), f32,
                                            name=f"ps_o{b}_{qb}_{sub}_{nn}",
                                            tag="o", bufs=O_BUFS)
                            nc.tensor.matmul(
                                ps_o[:],
                                lhsT=cx[:, sub * 128:(sub + 1) * 128],
                                rhs=wo_sb[:, nn * 512:(nn + 1) * 512],
                            )
                            nc.vector.tensor_copy(
                                o_sb[:, nn * 512:(nn + 1) * 512], ps_o[:])
                        r0 = g0 + sub * 128
                        nc.sync.dma_start(out[r0:r0 + 128, :], o_sb[:])

                return [stash, recips, bcast_mult,
                        lambda: outproj(0), lambda: outproj(1)]

            def emit_attn(b, qb, pend, blk):
                qb0 = qb * Q_W
                g0 = b * S + qb0
                n_t = (qb0 + Q_W) // 128  # causal: k-tiles needed
                ctxA = psp.tile((128, Q_W), f32, name=f"ctxA{b}_{qb}",
                                tag="ctx", bufs=CTX_BUFS)
                ctxB = psp.tile((128, Q_W), f32, name=f"ctxB{b}_{qb}",
                                tag="ctx", bufs=CTX_BUFS)
                ctxS = (psp.tile((128, Q_W), f32, name=f"ctxS{b}_{qb}",
                                 tag="ctx", bufs=CTX_BUFS) if C_SUM else ctxB)

                avq = []  # delayed AV args: (exp_sb, t, off, w)

                def emit_av(exp_sb, t, off, w):
                    first = t == 0
                    last = t == n_t - 1
                    kti = (b * S + 128 * t) // 128
                    nc.tensor.matmul(
                        ctxA[0:65, off:Q_W],
                        lhsT=V2_sb[:, kti, 0:65],
                        rhs=exp_sb[:, 0:w],
                        start=first,
                        stop=last,
                    )
                    nc.tensor.matmul(
                        ctxB[64:128, off:Q_W],
                        lhsT=V2_sb[:, kti, 65:129],
                        rhs=exp_sb[:, 512:512 + w],
                        start=first,
                        stop=last,
                        skip_group_check=True,
                    )
                    nc.tensor.matmul(
                        ctxS[0:1, off:Q_W],
                        lhsT=V2_sb[:, kti, 64:65],
                        rhs=exp_sb[:, 512:512 + w],
                        start=first,
                        stop=last,
                        skip_group_check=True,
                    )

                for t in range(n_t):
                    k0 = 128 * t
                    off = max(0, k0 - qb0)
                    w = Q_W - off
                    diag = k0 >= qb0
                    ps_s = psp.tile((128, 1024), f32,
                                    name=f"ps_s{b}_{qb}_{t}",
                                    tag="s", bufs=2)
                    exp_sb = sb.tile((128, 1024), bf16,
                                     name=f"exp{b}_{qb}_{t}",
                                     tag="exp", bufs=4)
                    for h in range(2):
                        nc.tensor.matmul(
                            ps_s[:, h * 512:h * 512 + w],
                            lhsT=KT_sb[h * 64:(h + 1) * 64,
                                       b * S + k0:b * S + k0 + 128],
                            rhs=QT_sb[h * 64:(h + 1) * 64,
                                      g0 + off:g0 + Q_W],
                            start=True,
                            stop=True,
                            tile_position=(h * 64, 0),
                            skip_group_check=True,
                        )
                    if t == 0 and pend:
                        pend[0]()  # stash(prev): frees ctx ring slots
                    if off == 0:
                        nc.scalar.activation(exp_sb[:, :1024], ps_s[:, :1024],
                                             Exp, scale=0.125)
                    elif C_EXP:
                        nc.scalar.activation(exp_sb[:, 0:w], ps_s[:, 0:w],
                                             Exp, scale=0.125)
                        nc.scalar.activation(exp_sb[:, 512:512 + w],
                                             ps_s[:, 512:512 + w],
                                             Exp, scale=0.125)
                    else:
                        s3 = ps_s[:].rearrange("p (h q) -> p h q",
                                               h=2)[:, :, 0:w]
                        e3 = exp_sb[:].rearrange("p (h q) -> p h q",
                                                 h=2)[:, :, 0:w]
                        nc.scalar.activation(e3, s3, Exp, scale=0.125)
                    if t == 0 and pend:
                        pend[1]()  # recips(prev): last ctx(prev) readers
                    if diag and C_MASK:
                        for h in range(2):
                            sb0 = h * 512
                            nc.vector.tensor_mul(
                                exp_sb[:, sb0:sb0 + 128],
                                exp_sb[:, sb0:sb0 + 128],
                                cmask[:],
                            )
                    elif diag:
                        em = exp_sb[:].rearrange("p (h q) -> p h q",
                                                 h=2)[:, :, 0:128]
                        nc.gpsimd.affine_select(
                            em, em,
                            pattern=[[0, 2], [1, 128]],
                            compare_op=is_ge,
                            fill=0.0,
                            base=0,
                            channel_multiplier=-1,
                        )
                    if t >= 1:
                        emit_av(*avq[t - 1])
                    avq.append((exp_sb, t, off, w))
                    if t == 1 and pend:
                        pend[2]()  # bcast+mult(prev)
                    if t == 2 and pend:
                        pend[3]()  # outproj(prev) subs 0-1
                    if t == 3 and pend:
                        pend[4]()  # outproj(prev) subs 2-3
                emit_av(*avq[n_t - 1])
                return make_tail(b, qb, ctxA, ctxB, ctxS, blk)

            # ---- interleaved emission --------------------------------------
            done_ch = 0
            pend = []
            for b in range(B):
                for qb in range(NQB):
                    need = b * NQB + qb + 1
                    while done_ch < need:
                        emit_qkv(done_ch)
                        done_ch += 1
                    pend = emit_attn(b, qb, pend, b * NQB + qb)
            while done_ch < N_CH:
                emit_qkv(done_ch)
                done_ch += 1
            for step in pend:
                step()

        if repeat == 1:
            body()
        else:
            with tc.For_i(0, repeat, 1):
                body()

    nc.compile()
    return nc


def _prep_inputs(x, Wq, bq, Wk, bk, Wv, bv, Wo):
    """Build the 8 per-core input maps (host-side sharding)."""
    x = np.asarray(x, dtype=np.float32)
    xT = x.reshape(BS, DM).T.astype(BF16)  # (DM, BS)
    # (chunk, 128, feat-tile, 512): xTc[ch, p, t, q] = xT[t*128+p, ch*512+q]
    xTc = np.ascontiguousarray(
        xT.reshape(KT_FEAT, 128, N_CH, Q_W).transpose(2, 1, 0, 3))

    def _w(W, sl):  # (128, feat-tile, CPC): w[p, t, m] = W[sl][m, t*128+p]
        wT = np.asarray(W, np.float32)[sl, :].T.astype(BF16)  # (DM, CPC)
        return np.ascontiguousarray(
            wT.reshape(KT_FEAT, 128, CPC).transpose(1, 0, 2))

    in_maps = []
    for c in range(NCORES):
        sl = slice(c * CPC, (c + 1) * CPC)
        in_maps.append({
            "xT": xTc,
            "wq": _w(Wq, sl),
            "wk": _w(Wk, sl),
            "wv": _w(Wv, sl),
            "wo": np.ascontiguousarray(np.asarray(Wo, np.float32)[:, sl].T).astype(BF16),
            "bq": np.asarray(bq, np.float32)[sl].reshape(CPC, 1).copy(),
            "bk": np.asarray(bk, np.float32)[sl].reshape(CPC, 1).copy(),
            "bv": np.asarray(bv, np.float32)[sl].reshape(CPC, 1).copy(),
        })
    return in_maps


def _run(in_maps, trace=False):
    global LAST_EXEC_NS, LAST_RESULTS
    from concourse import bass_utils

    if "nc" not in _CACHE:
        _CACHE["nc"] = _build()
    nc = _CACHE["nc"]
    res = bass_utils.run_bass_kernel_spmd(
        nc, in_maps, core_ids=list(range(NCORES)), trace=trace,
    )
    LAST_EXEC_NS = getattr(res, "exec_time_ns", None)
    LAST_RESULTS = res
    return res.results


def kernel(x, Wq, bq, Wk, bk, Wv, bv, Wo, bo, attention_mask=None, _trace=False):
    """Full inputs in, full output out. attention_mask is the reference's
    causal mask; causality is hardcoded in the kernel."""
    in_maps = _prep_inputs(x, Wq, bq, Wk, bk, Wv, bv, Wo)
    results = _run(in_maps, trace=_trace)
    acc = np.zeros((BS, DM), dtype=np.float32)
    for c in range(NCORES):
        acc += results[c]["out"].astype(np.float32)
    acc += np.asarray(bo, np.float32)[None, :]
    return acc.reshape(B, S, DM)
